# revision 1
# baseline (speedup 1.0000x reference)
"""Trainium2 Bass kernel for nn_CDEPdeLayerR2 (CDE PDE layer).

Pipeline per image:  convection (per-channel bilinear shift, replicate clamp)
-> fractional dilation (49-tap max-plus stencil) -> fractional erosion
(49-tap min-plus stencil) -> 32x32 channel mix (matmul).

Distribution: pure data parallel over 8 NeuronCores; batch 16 -> 2 images per
core.  Device layout: partition p = g*32 + ci  (g = horizontal slab of H/4
rows, ci = channel).  The channel-mix matmul uses a block-diagonal [128,128]
weight so no layout change is needed between the stencils and the matmul.

Engines: DVE runs the 2-elems/cycle fp16 tensor_tensor min/max chain; ACT
(ScalarE) computes `u_shifted + k_tap` (per-partition bias) for most taps; PE
does the matmul; stencil shifts are free-dim AP offsets; the per-channel
integer convection shift is folded into per-(image,channel,slab) DMA
descriptors reading a host edge-padded copy of x (replicate clamp baked in).
"""
import numpy as np

ALPHA = 0.65
KR = 3  # kernel radius
BIG = 60000.0  # fp16-safe stand-in for +inf
NCORES = 8
B, C, H, W = 16, 32, 256, 256
G = 4  # slabs per image -> 128 partitions

_PROGRAM_CACHE = {}
RUN_KWARGS = {}      # extra kwargs for run_bass_kernel_spmd (e.g. trace=True)
LAST_RESULT = None   # BassKernelResults of the most recent run


# ----------------------------------------------------------------- host math
def _morph_kernel_np(params):
    """Replicates reference.morphological_kernel in float64 -> [C,7,7] f32."""
    coords = np.arange(-KR, KR + 1, dtype=np.float64)
    yy = coords[:, None]
    xx = coords[None, :]
    rho = np.sqrt(xx * xx + yy * yy)
    theta = np.arctan2(yy + 0 * xx, xx + 0 * yy)
    K = params.shape[1]
    ks = np.arange(1, K + 1)
    freqs = ((ks + 1) // 2).astype(np.float64)
    ang = freqs[:, None, None] * theta[None]
    basis = np.where((ks % 2 == 1)[:, None, None], np.cos(ang), np.sin(ang))
    F = rho[None] * np.exp(
        -np.einsum("ck,kij->cij", params.astype(np.float64), basis))
    p = 2.0 * ALPHA / (2.0 * ALPHA - 1.0)
    nu = (2.0 * ALPHA - 1.0) * (2.0 * ALPHA) ** (-p)
    return (nu * F ** p).astype(np.float32)


def _conv_consts(c):
    cx = c[:, 0].astype(np.float64)
    cy = c[:, 1].astype(np.float64)
    ay = np.floor(-cy).astype(np.int64)
    ax = np.floor(-cx).astype(np.int64)
    wy = (-cy - ay).astype(np.float32)
    wx = (-cx - ax).astype(np.float32)
    return ax, ay, wx, wy


def _reference_np(x, c, kd, ke, weight):
    """Exact numpy fallback (slow) for pathological inputs."""
    Bs, Cs, Hs, Ws = x.shape
    ax, ay, wx, wy = _conv_consts(c)
    yi = np.arange(Hs)
    xi = np.arange(Ws)
    u = np.empty_like(x)
    for ci in range(Cs):
        y0 = np.clip(yi + ay[ci], 0, Hs - 1)
        y1 = np.clip(y0 + 1, 0, Hs - 1)
        x0 = np.clip(xi + ax[ci], 0, Ws - 1)
        x1 = np.clip(x0 + 1, 0, Ws - 1)
        a = x[:, ci][:, y0][:, :, x0]
        bq = x[:, ci][:, y0][:, :, x1]
        cq = x[:, ci][:, y1][:, :, x0]
        d = x[:, ci][:, y1][:, :, x1]
        u[:, ci] = ((1 - wy[ci]) * (1 - wx[ci]) * a + (1 - wy[ci]) * wx[ci] * bq
                    + wy[ci] * (1 - wx[ci]) * cq + wy[ci] * wx[ci] * d)

    def morph(v, k):
        big = 1e30
        pad = np.pad(v, ((0, 0), (0, 0), (KR, KR), (KR, KR)),
                     constant_values=big)
        out = np.full_like(v, big)
        for dy in range(2 * KR + 1):
            for dx in range(2 * KR + 1):
                cand = pad[:, :, dy:dy + Hs, dx:dx + Ws] + k[:, dy, dx][None, :, None, None]
                out = np.minimum(out, cand)
        return out

    u = -morph(-u, kd)
    u = morph(u, ke)
    return np.einsum("bihw,io->bohw", u, weight).astype(np.float32)


# -------------------------------------------------------------- bass program
def _build_program(ax, ay, b_local, h, w, n_dve_pure=15):
    import concourse.bacc as bacc
    import concourse.tile as tile
    import concourse.mybir as mybir

    fp16 = mybir.dt.float16
    f32 = mybir.dt.float32
    alu = mybir.AluOpType

    SL = h // G
    WP = w + 6
    SC = w + 2
    S0R = SL + 13          # S0 rows
    U0R = SL + 12          # u0 rows
    DR = SL + 6            # dil rows
    FDS = S0R * SC
    FDB = U0R * SC
    FDU = U0R * WP
    FDD = DR * WP
    FDE = SL * WP
    SLK = 8                # front/tail slack (elems) on u0/dil/ero tiles

    taps = [(dy, dx) for dy in range(-KR, KR + 1) for dx in range(-KR, KR + 1)]

    def tap_off(dy, dx):  # flat offset into (slacked) source tile
        return SLK + (dy + KR) * WP + dx

    init_tap = (-KR, 0)
    rest = [t for t in taps if t != init_tap]
    even_rest = [t for t in rest if tap_off(*t) % 2 == 0]
    odd_rest = [t for t in rest if tap_off(*t) % 2 == 1]
    n_pure = min(n_dve_pure, len(even_rest))
    pure_set = set(even_rest[:n_pure])
    act_list = even_rest[n_pure:] + odd_rest
    # Two pure-DVE taps first (ACT gets a head start on its adds), then a
    # Bresenham interleave so ACT never falls far behind the DVE min chain
    lead = 0
    order = list(even_rest[:lead])
    ai, di, accu = 0, lead, 0.0
    step = (n_pure - lead) / max(1, len(act_list))
    while ai < len(act_list) or di < n_pure:
        if ai < len(act_list):
            order.append(act_list[ai]); ai += 1
            accu += step
            if accu >= 1.0 and di < n_pure:
                order.append(even_rest[di]); di += 1
                accu -= 1.0
        elif di < n_pure:
            order.append(even_rest[di]); di += 1

    nc = bacc.Bacc("TRN2", target_bir_lowering=False, debug=False,
                   num_devices=NCORES)
    # pre-gathered S0 layout: [b, p=g*32+ci, S0R*SC] (shift+clamp baked on host)
    xh = nc.dram_tensor("xh", [b_local, 128, FDS], fp16,
                        kind="ExternalInput").ap()
    cv = nc.dram_tensor("cv", [128, 104], f32, kind="ExternalInput").ap()
    wb = nc.dram_tensor("wb", [128, 128], fp16, kind="ExternalInput").ap()
    out = nc.dram_tensor("out", [b_local, C, h, w], f32,
                         kind="ExternalOutput").ap()
    # [b, g, co, (rows*w)] view of the output for per-g PSUM stores
    out_r = out.rearrange("b co (g rn) w -> b g co (rn w)", g=G)

    with tile.TileContext(nc) as tc:
        with (
            tc.tile_pool(name="consts", bufs=1) as cpool,
            tc.tile_pool(name="big", bufs=1) as bigpool,
            tc.tile_pool(name="tmp", bufs=3) as tmppool,
            tc.tile_pool(name="obuf", bufs=2) as obufpool,
            tc.tile_pool(name="psum", bufs=4, space="PSUM") as psumpool,
        ):
            cv_sb = cpool.tile([128, 104], f32)
            nc.sync.dma_start(cv_sb[:], cv[:])
            wb_sb = cpool.tile([128, 128], fp16)
            nc.sync.dma_start(wb_sb[:], wb[:])
            ap_wy = cv_sb[:, 0:1]
            ap_1wy = cv_sb[:, 1:2]
            ap_wx = cv_sb[:, 2:3]
            ap_1wx = cv_sb[:, 3:4]

            def kcol(stage, t):  # stage 0: -kd, stage 1: +ke
                i = taps.index(t)
                return cv_sb[:, 4 + 49 * stage + i:5 + 49 * stage + i]

            HALF_D = (FDD // 2 + 3) & ~3
            HALF_E = (FDE // 2 + 3) & ~3
            HALF_MAX = max(HALF_D, FDD - HALF_D, HALF_E, FDE - HALF_E)

            for b in range(b_local):
                # ---------------- S0 load (host pre-gathered) ----
                # split so the first blend piece can start early
                S0 = bigpool.tile([128, FDS], fp16, tag="s0u0")
                S0v = S0[:, :].rearrange("p (r c) -> p r c", c=SC)
                scut = 17 * SC
                nc.sync.dma_start(S0[:, 0:scut], xh[b, :, 0:scut])
                nc.sync.dma_start(S0[:, scut:FDS], xh[b, :, scut:FDS])

                # ---------------- y blend ----------------
                # piece 1 = the top rows the By-fix needs, so the fix DMAs
                # overlap the remaining blend pieces
                By = bigpool.tile([128, FDB], fp16, tag="by")
                Byv = By[:, :].rearrange("p (r c) -> p r c", c=SC)
                j0max = int(min(max(0, 6 - ay.min()) + 1, U0R))
                rcuts = [0, j0max, U0R // 2, U0R]
                for rs, re_ in zip(rcuts, rcuts[1:]):
                    if rs >= re_:
                        continue
                    s, e = rs * SC, re_ * SC
                    nc.scalar.mul(By[:, s:e], S0[:, s:e], ap_1wy)
                    nc.vector.scalar_tensor_tensor(
                        By[:, s:e], S0[:, SC + s:SC + e], ap_wy, By[:, s:e],
                        alu.mult, alu.add)
                    if rs == 0:
                        for ci in range(C):
                            if ay[ci] <= -1:  # replicate-clamp y1 fix (g=0)
                                j0 = int(6 - ay[ci])
                                src = Byv[ci:ci + 1, j0:j0 + 1, :]
                                nc.sync.dma_start(
                                    Byv[ci:ci + 1, 0:j0, :],
                                    src.broadcast_to([1, j0, SC]))

                # ---------------- x blend ----------------
                # left column strip first; x1-fix DMAs overlap the rest
                u0 = bigpool.tile([128, SLK + FDU + SLK], fp16, tag="s0u0")
                u0v = u0[:, SLK:SLK + FDU].rearrange("p (r c) -> p r c", c=WP)
                ccuts = [0, 64, w // 2 + 32, w]
                for cs, ce in zip(ccuts, ccuts[1:]):
                    u0_real = u0v[:, :, 3 + cs:3 + ce]
                    nc.scalar.mul(u0_real, Byv[:, :, cs:ce], ap_1wx)
                    nc.vector.scalar_tensor_tensor(
                        u0_real, Byv[:, :, 1 + cs:1 + ce], ap_wx, u0_real,
                        alu.mult, alu.add)
                    if cs == 0:
                        nfix = 0
                        for ci in range(C):
                            if ax[ci] <= -1:  # replicate-clamp x1 fix (left)
                                cc0 = int(-ax[ci])
                                for g in range(G):
                                    p = g * 32 + ci
                                    for cc in range(cc0):
                                        eng = (nc.sync if nfix % 2 == 0
                                               else nc.gpsimd)
                                        nfix += 1
                                        eng.dma_start(
                                            u0v[p:p + 1, :, 3 + cc:4 + cc],
                                            u0v[p:p + 1, :, 3 + cc0:4 + cc0])
                # -BIG pads (+ slack so edge-tap reads are initialized);
                # on DVE so they sit inline in the critical stream (no
                # cross-engine stall)
                nc.vector.memset(u0[:, 0:SLK], -BIG)
                nc.vector.memset(u0[:, SLK + FDU:SLK + FDU + SLK], -BIG)
                nc.vector.memset(u0v[:, :, 0:3], -BIG)
                nc.vector.memset(u0v[:, :, 3 + w:WP], -BIG)
                nc.vector.memset(u0v[0:32, 0:6, :], -BIG)
                nc.vector.memset(u0v[96:128, U0R - 6:U0R, :], -BIG)

                # ---------------- stencils ----------------
                # Two independent accumulator chains per stage: chain A is
                # fed by ACT tap-adds, chain B is pure-DVE — so DVE never
                # stalls on ACT; merged at the end.  Half-major order so the
                # first half of the erosion output is ready early for PE.
                def stencil(src_t, fd_out, stage, op):
                    acc = bigpool.tile(
                        [128, SLK + fd_out + SLK], fp16,
                        tag="accd" if stage == 0 else "acce")
                    accb = bigpool.tile([128, fd_out], fp16, tag="by")
                    accf = acc[:, SLK:SLK + fd_out]
                    half = HALF_D if stage == 0 else HALF_E
                    bounds = [(0, half), (half, fd_out)]
                    a_init, b_init = act_list[0], init_tap
                    for s, e in bounds:
                        oa = tap_off(*a_init)
                        obt = tap_off(*b_init)
                        nc.vector.tensor_scalar_add(
                            accf[:, s:e], src_t[:, oa + s:oa + e],
                            kcol(stage, a_init))
                        nc.vector.tensor_scalar_add(
                            accb[:, s:e], src_t[:, obt + s:obt + e],
                            kcol(stage, b_init))
                        for t in order:
                            if t == a_init:
                                continue
                            o = tap_off(*t)
                            if t in pure_set:
                                tmp = tmppool.tile([128, HALF_MAX], fp16,
                                                   tag="tmp")
                                tv = tmp[:, 0:e - s]
                                nc.vector.tensor_scalar_add(
                                    tv, src_t[:, o + s:o + e], kcol(stage, t))
                                nc.vector.tensor_tensor(
                                    accb[:, s:e], tv, accb[:, s:e], op)
                            else:
                                tmp = tmppool.tile([128, HALF_MAX], fp16,
                                                   tag="tmp")
                                tv = tmp[:, 0:e - s]
                                nc.scalar.add(
                                    tv, src_t[:, o + s:o + e], kcol(stage, t))
                                nc.vector.tensor_tensor(
                                    accf[:, s:e], tv, accf[:, s:e], op)
                        nc.vector.tensor_tensor(
                            accf[:, s:e], accb[:, s:e], accf[:, s:e], op)
                    return acc

                dil = stencil(u0, FDD, 0, alu.max)
                dv = dil[:, SLK:SLK + FDD].rearrange("p (r c) -> p r c", c=WP)
                nc.vector.memset(dil[:, 0:SLK], BIG)
                nc.vector.memset(dil[:, SLK + FDD:SLK + FDD + SLK], BIG)
                nc.vector.memset(dv[:, :, 0:3], BIG)
                nc.vector.memset(dv[:, :, 3 + w:WP], BIG)
                nc.vector.memset(dv[0:32, 0:3, :], BIG)
                nc.vector.memset(dv[96:128, DR - 3:DR, :], BIG)

                ero = stencil(dil, FDE, 1, alu.min)
                ev = ero[:, SLK:SLK + FDE].rearrange("p (r c) -> p r c", c=WP)

                # ---------------- channel mix + store ----------------
                rows_per_mm = 512 // w  # 2
                for k in range(SL // rows_per_mm):
                    ps = psumpool.tile([128, rows_per_mm * w], f32)
                    nc.tensor.matmul(
                        ps[:], wb_sb[:],
                        ev[:, k * rows_per_mm:(k + 1) * rows_per_mm, 3:3 + w],
                        start=True, stop=True)
                    ob = obufpool.tile([128, rows_per_mm * w], f32, tag="ob")
                    nc.scalar.copy(ob[:], ps[:])
                    nn = rows_per_mm * w
                    for g in range(G):
                        # store triggers ride the (mostly idle) GpSimd queue
                        nc.gpsimd.dma_start(
                            out_r[b, g, :, k * nn:(k + 1) * nn],
                            ob[g * 32:(g + 1) * 32, :])

    nc.compile()
    return nc


# ------------------------------------------------------------------- kernel
def kernel(x, c, finsler_dil, finsler_ero, weight):
    x = np.ascontiguousarray(np.asarray(x, dtype=np.float32))
    c = np.asarray(c, dtype=np.float32)
    weight = np.asarray(weight, dtype=np.float32)
    kd = _morph_kernel_np(np.asarray(finsler_dil, dtype=np.float32))
    ke = _morph_kernel_np(np.asarray(finsler_ero, dtype=np.float32))

    ax, ay, wx, wy = _conv_consts(c)
    amax = float(np.abs(x).max())
    if amax > 10000.0 or np.abs(ax).max() > 50 or np.abs(ay).max() > 50:
        return _reference_np(x, c, kd, ke, weight)

    kclamp = min(25000.0, max(1000.0, 2.2 * amax + 10.0))
    kd = np.minimum(kd, kclamp)
    ke = np.minimum(ke, kclamp)

    key = (tuple(ax.tolist()), tuple(ay.tolist()), x.shape)
    if key not in _PROGRAM_CACHE:
        _PROGRAM_CACHE[key] = _build_program(ax, ay, B // NCORES, H, W)
    nc = _PROGRAM_CACHE[key]

    # per-partition constants: p = g*32 + ci
    cv = np.zeros((128, 104), np.float32)
    rep = np.tile(np.arange(C), G)
    cv[:, 0] = wy[rep]
    cv[:, 1] = 1.0 - wy[rep]
    cv[:, 2] = wx[rep]
    cv[:, 3] = 1.0 - wx[rep]
    cv[:, 4:53] = (-kd.reshape(C, 49))[rep]
    cv[:, 53:102] = ke.reshape(C, 49)[rep]

    wblk = np.zeros((128, 128), np.float16)
    for g in range(G):
        wblk[g * 32:(g + 1) * 32, g * 32:(g + 1) * 32] = weight.astype(np.float16)

    # host gather into the device S0 layout: xg[b, p=g*32+ci, j, cc] =
    # x[b, ci, clip(g*SL-6+j+ay_ci), clip(cc+ax_ci)]
    SL = H // G
    SC = W + 2
    S0R = SL + 13
    x16 = x.astype(np.float16)
    xg = np.empty((B, G * C, S0R, SC), np.float16)
    jj = np.arange(S0R)
    cc = np.arange(SC)
    for ci in range(C):
        rows = np.clip(jj[None, :] + (np.arange(G) * SL)[:, None] - 6 + int(ay[ci]),
                       0, H - 1)                       # [G, S0R]
        cols = np.clip(cc + int(ax[ci]), 0, W - 1)     # [SC]
        xg[:, ci::C] = x16[:, ci][:, rows][:, :, :, cols]
    xg = xg.reshape(B, 128, S0R * SC)

    bl = B // NCORES
    in_maps = [
        {"xh": xg[i * bl:(i + 1) * bl], "cv": cv, "wb": wblk}
        for i in range(NCORES)
    ]

    from concourse.bass_utils import run_bass_kernel_spmd
    res = run_bass_kernel_spmd(nc, in_maps, core_ids=list(range(NCORES)),
                               **RUN_KWARGS)
    global LAST_RESULT
    LAST_RESULT = res
    return np.concatenate([res.results[i]["out"] for i in range(NCORES)], axis=0)



# revision 3
# speedup vs baseline: 1.3541x; 1.3541x over previous
"""Trainium2 Bass kernel for nn_CDEPdeLayerR2 (CDE PDE layer).

Pipeline per image:  convection (per-channel bilinear shift, replicate clamp)
-> fractional dilation (49-tap max-plus stencil) -> fractional erosion
(49-tap min-plus stencil) -> 32x32 channel mix (matmul).

Distribution: pure data parallel over 8 NeuronCores; batch 16 -> 2 images per
core.  Device layout: partition p = g*32 + ci  (g = horizontal slab of H/4
rows, ci = channel).  The channel-mix matmul uses a block-diagonal [128,128]
weight so no layout change is needed between the stencils and the matmul.

Every stencil tap is ONE fused DVE instruction acc = max/min(src + k, acc)
via custom-authored DVE micro-ops that include a hand-built 2X_1PORT
(2 elem/cycle fp16) program — upstream bass only lowers 1x custom ops.
Odd-offset taps read a 1-element-shifted copy of the source tile (made on
the otherwise-idle ACT engine) so every operand stays 4-byte aligned and the
2x mode engages.  The bilinear convection blends use a fused mul-add custom
op the same way.  The per-channel integer convection shift + replicate clamp
is baked into per-(image,channel,slab) host gathers of x.
"""
import numpy as np

ALPHA = 0.65
KR = 3  # kernel radius
BIG = 60000.0  # fp16-safe stand-in for +inf
NCORES = 8
B, C, H, W = 16, 32, 256, 256
G = 4  # slabs per image -> 128 partitions

_PROGRAM_CACHE = {}
RUN_KWARGS = {}      # extra kwargs for run_bass_kernel_spmd (e.g. trace=True)
LAST_RESULT = None   # BassKernelResults of the most recent run
_CUSTOM_OPS = {}


# ----------------------------------------------------------------- host math
def _morph_kernel_np(params):
    """Replicates reference.morphological_kernel in float64 -> [C,7,7] f32."""
    coords = np.arange(-KR, KR + 1, dtype=np.float64)
    yy = coords[:, None]
    xx = coords[None, :]
    rho = np.sqrt(xx * xx + yy * yy)
    theta = np.arctan2(yy + 0 * xx, xx + 0 * yy)
    K = params.shape[1]
    ks = np.arange(1, K + 1)
    freqs = ((ks + 1) // 2).astype(np.float64)
    ang = freqs[:, None, None] * theta[None]
    basis = np.where((ks % 2 == 1)[:, None, None], np.cos(ang), np.sin(ang))
    F = rho[None] * np.exp(
        -np.einsum("ck,kij->cij", params.astype(np.float64), basis))
    p = 2.0 * ALPHA / (2.0 * ALPHA - 1.0)
    nu = (2.0 * ALPHA - 1.0) * (2.0 * ALPHA) ** (-p)
    return (nu * F ** p).astype(np.float32)


def _conv_consts(c):
    cx = c[:, 0].astype(np.float64)
    cy = c[:, 1].astype(np.float64)
    ay = np.floor(-cy).astype(np.int64)
    ax = np.floor(-cx).astype(np.int64)
    wy = (-cy - ay).astype(np.float32)
    wx = (-cx - ax).astype(np.float32)
    return ax, ay, wx, wy


def _reference_np(x, c, kd, ke, weight):
    """Exact numpy fallback (slow) for pathological inputs."""
    Bs, Cs, Hs, Ws = x.shape
    ax, ay, wx, wy = _conv_consts(c)
    yi = np.arange(Hs)
    xi = np.arange(Ws)
    u = np.empty_like(x)
    for ci in range(Cs):
        y0 = np.clip(yi + ay[ci], 0, Hs - 1)
        y1 = np.clip(y0 + 1, 0, Hs - 1)
        x0 = np.clip(xi + ax[ci], 0, Ws - 1)
        x1 = np.clip(x0 + 1, 0, Ws - 1)
        a = x[:, ci][:, y0][:, :, x0]
        bq = x[:, ci][:, y0][:, :, x1]
        cq = x[:, ci][:, y1][:, :, x0]
        d = x[:, ci][:, y1][:, :, x1]
        u[:, ci] = ((1 - wy[ci]) * (1 - wx[ci]) * a + (1 - wy[ci]) * wx[ci] * bq
                    + wy[ci] * (1 - wx[ci]) * cq + wy[ci] * wx[ci] * d)

    def morph(v, k):
        big = 1e30
        pad = np.pad(v, ((0, 0), (0, 0), (KR, KR), (KR, KR)),
                     constant_values=big)
        out = np.full_like(v, big)
        for dy in range(2 * KR + 1):
            for dx in range(2 * KR + 1):
                cand = pad[:, :, dy:dy + Hs, dx:dx + Ws] + k[:, dy, dx][None, :, None, None]
                out = np.minimum(out, cand)
        return out

    u = -morph(-u, kd)
    u = morph(u, ke)
    return np.einsum("bihw,io->bohw", u, weight).astype(np.float32)


# ----------------------------------------------------- custom fused DVE ops
def _ensure_custom_ops():
    """Register fused ops acc=max/min(src+k, acc) and out=src*s+acc with a
    hand-authored 2X_1PORT microprogram (upstream lower() emits 1x only).
    Uses dve_ops' documented extension point (append to OPS) and pre-seeds
    the compile cache with a DveOpSpec carrying uops_2x + perf_max."""
    if _CUSTOM_OPS:
        return _CUSTOM_OPS
    from concourse import dve_ops
    from concourse.dve_spec import Spec, Src0, Src1, C0, maxx, minn, lower
    from concourse.dve_uop import (
        DveOpSpec, UopConfig, InpSel, OutSel, OutPath, AluInp, AluOp,
        DelayInp, Trigger, ENABLE,
    )

    def build_2x(alu0, alu1):
        """2X_1PORT program for out = alu1(alu0(Src0, C0), Src1), packed fp16:
        one 32-bit word = 2 elems/port/cycle; lo pair through blocks 0-1, hi
        pair through blocks 2-3 (lo result rides delay lane 0)."""
        u = UopConfig()
        u.enable_input(InpSel.SRC_0, 1)
        u.enable_input(InpSel.CONST_0, 2)
        u.enable_input(InpSel.SRC_1, 3)
        u.enable_input(InpSel.SRC_0_HI, 4)
        u.enable_input(InpSel.SRC_1_HI, 5)
        u.require_inp0 = ENABLE
        u.require_inp1 = ENABLE
        u.trigger = (Trigger.SRC_TENSOR_DONE, Trigger.NONE, Trigger.NONE)
        dp = u.datapath_config
        dp[0].enable_alu(alu0, AluInp.PREV_DELAY_0, AluInp.PREV_DELAY_1)
        dp[0].pass_through_delay(1, 2, 3, 4)
        dp[1].enable_alu(alu1, AluInp.PREV_ALU_OUT, AluInp.PREV_DELAY_2)
        dp[1].pass_through_delay(1, 3, 4)
        dp[2].enable_alu(alu0, AluInp.PREV_DELAY_3, AluInp.PREV_DELAY_1)
        dp[2].enable_delay_from_src(DelayInp.PREV_ALU_OUT, 0)
        dp[2].pass_through_delay(4)
        dp[3].enable_alu(alu1, AluInp.PREV_ALU_OUT, AluInp.PREV_DELAY_4)
        dp[3].pass_through_delay(0)
        for bi in range(4, 8):
            dp[bi].pass_through_alu()
            dp[bi].pass_through_delay(0)
        u.enable_output(OutSel.DELAY_0, OutPath.WR0_LO)
        u.enable_output(OutSel.ALU_OUT, OutPath.WR0_HI)
        return u

    def register(name, alu0, alu1, body, ref):
        if name in dve_ops._SUB_OPCODE_FOR_NAME:
            return next(o for o in dve_ops.OPS if o.name == name)
        row = max(dve_ops._SUB_OPCODE_FOR_NAME.values()) + 1
        assert row < 0x20, "no free custom-DVE opcode rows"
        op = dve_ops.DveOp(name, Spec(body=body, reference=ref),
                           subdim=False, uops_sha={})
        spec2 = DveOpSpec(name=name, opcode=row, uops=lower(op.spec, ver="v3"),
                          uops_2x=[build_2x(alu0, alu1)], perf_max=1,
                          rd1_en=True)
        spec2.validate("v3")
        dve_ops.OPS.append(op)
        dve_ops.CUSTOM_DVE_SPECS[name] = op.spec
        dve_ops._SUB_OPCODE_FOR_NAME[name] = row
        dve_ops._COMPILE_CACHE[(name, "v3")] = spec2
        return op

    from concourse.dve_uop import AluOp as _A
    _CUSTOM_OPS["tap_max"] = register(
        "TAP_MAX_ANT", _A.ADD, _A.MAX, maxx(Src0 + C0, Src1),
        lambda in0, in1, s0, s1, imm2: np.maximum(
            in0.astype(np.float32) + s0, in1))
    _CUSTOM_OPS["tap_min"] = register(
        "TAP_MIN_ANT", _A.ADD, _A.MIN, minn(Src0 + C0, Src1),
        lambda in0, in1, s0, s1, imm2: np.minimum(
            in0.astype(np.float32) + s0, in1))
    _CUSTOM_OPS["muladd"] = register(
        "MULADD2X_ANT", _A.MULTIPLY, _A.ADD, Src0 * C0 + Src1,
        lambda in0, in1, s0, s1, imm2: in0.astype(np.float32) * s0 + in1)
    return _CUSTOM_OPS


def _emit(vec, op, out, in0, in1, s0):
    """vec._custom_dve with perf_max=1 (stock emitter hardwires 0 = 1x)."""
    inst = vec._custom_dve(op, out=out, in0=in0, in1=in1, s0=s0)
    inst.ins.perf_max = 1  # BassInstruction wraps the rust inst at .ins
    return inst


# -------------------------------------------------------------- bass program
def _build_program(ax, ay, b_local, h, w):
    import concourse.bacc as bacc
    import concourse.tile as tile
    import concourse.mybir as mybir

    ops = _ensure_custom_ops()
    fp16 = mybir.dt.float16
    f32 = mybir.dt.float32

    SL = h // G            # 64 slab rows
    WP = w + 6             # padded row width (4 left pad, 2 right pad)
    SC = w + 2             # gathered source cols
    S0R = SL + 13
    U0R = SL + 12
    DR = SL + 6
    FDS = S0R * SC
    FDB = U0R * SC
    FDU = U0R * WP
    FDD = DR * WP
    FDE = SL * WP
    SLK = 8                # front/tail slack (elems), keeps edge reads defined
    SZU = SLK + FDU + SLK
    SZD = SLK + FDD + SLK
    SZE = SLK + FDE + SLK

    taps = [(dy, dx) for dy in range(-KR, KR + 1) for dx in range(-KR, KR + 1)]

    def tap_off(dy, dx):   # flat offset of tap read vs acc position
        return (dy + KR) * WP + dx

    even_taps = [t for t in taps if tap_off(*t) % 2 == 0]
    odd_taps = [t for t in taps if tap_off(*t) % 2 == 1]
    init_tap = even_taps[0]

    nc = bacc.Bacc("TRN2", target_bir_lowering=False, debug=False,
                   num_devices=NCORES)
    # pre-gathered S0 layout: [b, p=g*32+ci, S0R*SC] (shift+clamp baked on host)
    xh = nc.dram_tensor("xh", [b_local, 128, FDS], fp16,
                        kind="ExternalInput").ap()
    cv = nc.dram_tensor("cv", [128, 104], f32, kind="ExternalInput").ap()
    wb = nc.dram_tensor("wb", [128, 128], fp16, kind="ExternalInput").ap()
    out = nc.dram_tensor("out", [b_local, C, h, w], f32,
                         kind="ExternalOutput").ap()
    out_r = out.rearrange("b co (g rn) w -> b g co (rn w)", g=G)

    with tile.TileContext(nc) as tc:
        with (
            tc.tile_pool(name="consts", bufs=1) as cpool,
            tc.tile_pool(name="big", bufs=1) as bigpool,
            tc.tile_pool(name="obuf", bufs=2) as obufpool,
            tc.tile_pool(name="psum", bufs=4, space="PSUM") as psumpool,
        ):
            cv_sb = cpool.tile([128, 104], f32)
            nc.sync.dma_start(cv_sb[:], cv[:])
            wb_sb = cpool.tile([128, 128], fp16)
            nc.sync.dma_start(wb_sb[:], wb[:])
            ap_wy = cv_sb[:, 0:1]
            ap_1wy = cv_sb[:, 1:2]
            ap_wx = cv_sb[:, 2:3]
            ap_1wx = cv_sb[:, 3:4]

            def kcol(stage, t):  # stage 0: -kd, stage 1: +ke
                i = taps.index(t)
                return cv_sb[:, 4 + 49 * stage + i:5 + 49 * stage + i]

            for b in range(b_local):
                # ---------------- S0 load (host pre-gathered) ----
                S0 = bigpool.tile([128, FDS], fp16, tag="s0dil")
                S0v = S0[:, :].rearrange("p (r c) -> p r c", c=SC)
                scut = (FDS // 2) & ~1
                nc.sync.dma_start(S0[:, 0:scut], xh[b, :, 0:scut])
                nc.gpsimd.dma_start(S0[:, scut:FDS], xh[b, :, scut:FDS])

                # ---------------- y blend:  By = (1-wy)*S0 + wy*S0[+1 row]
                By = bigpool.tile([128, FDB], fp16, tag="by")
                Byv = By[:, :].rearrange("p (r c) -> p r c", c=SC)
                nc.scalar.mul(By[:, :], S0[:, 0:FDB], ap_1wy)
                _emit(nc.vector, ops["muladd"], By[:, :], S0[:, SC:SC + FDB],
                      By[:, :], ap_wy)
                for ci in range(C):
                    if ay[ci] <= -1:  # replicate-clamp y1 fix (g=0)
                        j0 = int(6 - ay[ci])
                        if j0 >= U0R:
                            j0 = U0R - 1
                        src = Byv[ci:ci + 1, j0:j0 + 1, :]
                        nc.sync.dma_start(
                            Byv[ci:ci + 1, 0:j0, :],
                            src.broadcast_to([1, j0, SC]))

                # ---------------- x blend:  u0 = (1-wx)*By + wx*By[+1 col]
                u0 = bigpool.tile([128, SZU], fp16, tag="u0")
                u0v = u0[:, SLK:SLK + FDU].rearrange("p (r c) -> p r c", c=WP)
                u0_real = u0v[:, :, 4:4 + w]
                nc.scalar.mul(u0_real, Byv[:, :, 1:1 + w], ap_wx)
                _emit(nc.vector, ops["muladd"], u0_real, Byv[:, :, 0:w],
                      u0_real, ap_1wx)
                nfix = 0
                for ci in range(C):
                    if ax[ci] <= -1:  # replicate-clamp x1 fix (left cols)
                        cc0 = min(int(-ax[ci]), w - 1)
                        for g in range(G):
                            p = g * 32 + ci
                            for cc in range(cc0):
                                eng = nc.sync if nfix % 2 == 0 else nc.gpsimd
                                nfix += 1
                                eng.dma_start(
                                    u0v[p:p + 1, :, 4 + cc:5 + cc],
                                    u0v[p:p + 1, :, 4 + cc0:5 + cc0])
                # -BIG pads (ACT; DVE is the bottleneck engine)
                nc.gpsimd.memset(u0[:, 0:SLK], -BIG)
                nc.gpsimd.memset(u0[:, SLK + FDU:SZU], -BIG)
                nc.gpsimd.memset(u0v[:, :, 0:4], -BIG)
                nc.gpsimd.memset(u0v[:, :, 4 + w:WP], -BIG)
                nc.gpsimd.memset(u0v[0:32, 0:6, :], -BIG)
                nc.gpsimd.memset(u0v[96:128, U0R - 6:U0R, :], -BIG)
                # shifted copy for odd-offset taps (ACT, off critical path)
                u0p = bigpool.tile([128, SZU], fp16, tag="shift")
                nc.scalar.copy(u0p[:, 0:SZU - 1], u0[:, 1:SZU])

                # ---------------- dilation: 49 fused max taps ----
                dil = bigpool.tile([128, SZD], fp16, tag="s0dil")
                dacc = dil[:, SLK:SLK + FDD]
                o0 = SLK + tap_off(*init_tap)
                nc.vector.tensor_scalar_add(dacc, u0[:, o0:o0 + FDD],
                                            kcol(0, init_tap))
                for t in even_taps[1:]:
                    o = SLK + tap_off(*t)
                    _emit(nc.vector, ops["tap_max"], dacc,
                          u0[:, o:o + FDD], dacc, kcol(0, t))
                for t in odd_taps:
                    o = SLK + tap_off(*t) - 1
                    _emit(nc.vector, ops["tap_max"], dacc,
                          u0p[:, o:o + FDD], dacc, kcol(0, t))
                dv = dil[:, SLK:SLK + FDD].rearrange("p (r c) -> p r c", c=WP)
                nc.gpsimd.memset(dil[:, 0:SLK], BIG)
                nc.gpsimd.memset(dil[:, SLK + FDD:SZD], BIG)
                nc.gpsimd.memset(dv[:, :, 0:4], BIG)
                nc.gpsimd.memset(dv[:, :, 4 + w:WP], BIG)
                nc.gpsimd.memset(dv[0:32, 0:3, :], BIG)
                nc.gpsimd.memset(dv[96:128, DR - 3:DR, :], BIG)
                dilp = bigpool.tile([128, SZD], fp16, tag="shift")
                nc.scalar.copy(dilp[:, 0:SZD - 1], dil[:, 1:SZD])

                # ---------------- erosion: 49 fused min taps ----
                # even taps (reading dil) first: frees the s0dil tag early so
                # the next image's S0 load overlaps the odd-tap phase
                ero = bigpool.tile([128, SZE], fp16, tag="ero")
                eacc = ero[:, SLK:SLK + FDE]
                nc.vector.tensor_scalar_add(eacc, dil[:, o0:o0 + FDE],
                                            kcol(1, init_tap))
                for t in even_taps[1:]:
                    o = SLK + tap_off(*t)
                    _emit(nc.vector, ops["tap_min"], eacc,
                          dil[:, o:o + FDE], eacc, kcol(1, t))
                for t in odd_taps:
                    o = SLK + tap_off(*t) - 1
                    _emit(nc.vector, ops["tap_min"], eacc,
                          dilp[:, o:o + FDE], eacc, kcol(1, t))
                ev = ero[:, SLK:SLK + FDE].rearrange("p (r c) -> p r c", c=WP)

                # ---------------- channel mix + store ----------------
                rows_per_mm = 512 // w  # 2
                for k in range(SL // rows_per_mm):
                    ps = psumpool.tile([128, rows_per_mm * w], f32)
                    nc.tensor.matmul(
                        ps[:], wb_sb[:],
                        ev[:, k * rows_per_mm:(k + 1) * rows_per_mm, 4:4 + w],
                        start=True, stop=True)
                    ob = obufpool.tile([128, rows_per_mm * w], f32, tag="ob")
                    nc.scalar.copy(ob[:], ps[:])
                    nn = rows_per_mm * w
                    for g in range(G):
                        nc.gpsimd.dma_start(
                            out_r[b, g, :, k * nn:(k + 1) * nn],
                            ob[g * 32:(g + 1) * 32, :])

    nc.compile()
    return nc


# ------------------------------------------------------------------- kernel
def kernel(x, c, finsler_dil, finsler_ero, weight):
    x = np.ascontiguousarray(np.asarray(x, dtype=np.float32))
    c = np.asarray(c, dtype=np.float32)
    weight = np.asarray(weight, dtype=np.float32)
    kd = _morph_kernel_np(np.asarray(finsler_dil, dtype=np.float32))
    ke = _morph_kernel_np(np.asarray(finsler_ero, dtype=np.float32))

    ax, ay, wx, wy = _conv_consts(c)
    amax = float(np.abs(x).max())
    if amax > 10000.0 or np.abs(ax).max() > 50 or np.abs(ay).max() > 50:
        return _reference_np(x, c, kd, ke, weight)

    kclamp = min(25000.0, max(1000.0, 2.2 * amax + 10.0))
    kd = np.minimum(kd, kclamp)
    ke = np.minimum(ke, kclamp)

    key = (tuple(ax.tolist()), tuple(ay.tolist()), x.shape)
    if key not in _PROGRAM_CACHE:
        _PROGRAM_CACHE[key] = _build_program(ax, ay, B // NCORES, H, W)
    nc = _PROGRAM_CACHE[key]

    # per-partition constants: p = g*32 + ci
    cv = np.zeros((128, 104), np.float32)
    rep = np.tile(np.arange(C), G)
    cv[:, 0] = wy[rep]
    cv[:, 1] = 1.0 - wy[rep]
    cv[:, 2] = wx[rep]
    cv[:, 3] = 1.0 - wx[rep]
    cv[:, 4:53] = (-kd.reshape(C, 49))[rep]
    cv[:, 53:102] = ke.reshape(C, 49)[rep]

    wblk = np.zeros((128, 128), np.float16)
    for g in range(G):
        wblk[g * 32:(g + 1) * 32, g * 32:(g + 1) * 32] = weight.astype(np.float16)

    # host gather into the device S0 layout: xg[b, p=g*32+ci, j, cc] =
    # x[b, ci, clip(g*SL-6+j+ay_ci), clip(cc+ax_ci)]
    SL = H // G
    SC = W + 2
    S0R = SL + 13
    x16 = x.astype(np.float16)
    xg = np.empty((B, G * C, S0R, SC), np.float16)
    jj = np.arange(S0R)
    cc = np.arange(SC)
    for ci in range(C):
        rows = np.clip(jj[None, :] + (np.arange(G) * SL)[:, None] - 6 + int(ay[ci]),
                       0, H - 1)                       # [G, S0R]
        cols = np.clip(cc + int(ax[ci]), 0, W - 1)     # [SC]
        xg[:, ci::C] = x16[:, ci][:, rows][:, :, :, cols]
    xg = xg.reshape(B, 128, S0R * SC)

    bl = B // NCORES
    in_maps = [
        {"xh": xg[i * bl:(i + 1) * bl], "cv": cv, "wb": wblk}
        for i in range(NCORES)
    ]

    from concourse.bass_utils import run_bass_kernel_spmd
    res = run_bass_kernel_spmd(nc, in_maps, core_ids=list(range(NCORES)),
                               **RUN_KWARGS)
    global LAST_RESULT
    LAST_RESULT = res
    return np.concatenate([res.results[i]["out"] for i in range(NCORES)], axis=0)
